# revision 10
# baseline (speedup 1.0000x reference)
"""Batched structure decoder: out[g] = sigmoid(z_g @ z_g^T), masked to valid nodes.

Full inputs in, full output out. Shards the 128 graphs across 8 NeuronCores
(16 graphs each); each core computes its own [16, 512, 512] block with no
cross-device communication.

Per-core device kernel (Bass/Tile):
  for each graph g (16 per core):
    - DMA z_g [512, 256] fp32 into SBUF as [128p, 4t, 256d]
    - 8x PE transpose (128x128 blocks) -> zT [128d, 2kt, 512n] via PSUM + DVE copy
    - 8x f32r matmul (4 m-tiles x 2 k-subtiles) into PSUM [128, 2048]
    - sigmoid on ScalarE PSUM -> SBUF
    - DMA out [512, 512] (1 MB contiguous)
"""

import numpy as np

import concourse.bass as bass
import concourse.tile as tile
from concourse import bacc, mybir
from concourse.bass_utils import run_bass_kernel_spmd
from concourse.masks import make_identity

NUM_GRAPHS = 128
MAX_NODES = 512
LATENT_DIM = 256
N_CORES = 8
G_PER_CORE = NUM_GRAPHS // N_CORES  # 16
P = 128
N_TILES = MAX_NODES // P  # 4 node tiles per graph
K_TILES = LATENT_DIM // P  # 2 contraction subtiles

_NC = None  # cached Bass program
_last_results = None  # BassKernelResults of the most recent run (for profiling)


def _build_bass():
    nc = bacc.Bacc("TRN2", target_bir_lowering=False)
    z = nc.dram_tensor(
        "z", (G_PER_CORE * MAX_NODES, LATENT_DIM), mybir.dt.float32,
        kind="ExternalInput",
    )
    out = nc.dram_tensor(
        "out", (G_PER_CORE, MAX_NODES, MAX_NODES), mybir.dt.float32,
        kind="ExternalOutput",
    )
    # z[g*512 + t*128 + p, d] -> [g, p, t, d]
    z_r = z[:].rearrange("(g t p) d -> g p t d", t=N_TILES, p=P)
    # out[g, 256*h + m*128 + p, n] -> [g, h, p, m, n]
    out_r = out[:].rearrange("g (h m p) n -> g h p m n", h=2, p=P)

    with tile.TileContext(nc) as tc:
        with (
            tc.tile_pool(name="singles", bufs=1) as singles,
            tc.tile_pool(name="zin", bufs=6) as zin_pool,
            tc.tile_pool(name="z32", bufs=2) as z32_pool,
            tc.tile_pool(name="zt", bufs=4) as zt_pool,
            tc.tile_pool(name="osb", bufs=8) as out_pool,
            tc.tile_pool(name="pst", bufs=2, space="PSUM") as psum_t_pool,
            tc.tile_pool(name="psmm", bufs=3, space="PSUM") as psum_mm_pool,
        ):
            identity = singles.tile([P, P], mybir.dt.float16)
            make_identity(nc, identity)

            for g in range(G_PER_CORE):
                z16 = zin_pool.tile([P, N_TILES, LATENT_DIM], mybir.dt.float16)
                if g < 2:
                    # Ramp bootstrap: the SWDGE (gpsimd) path has ~2.7us
                    # descriptor-emission latency per DMA; for the first two
                    # graphs use the idle sync HWDGE queue (fast start) and
                    # cast on GpSimd instead.
                    z32 = z32_pool.tile([P, N_TILES, LATENT_DIM], mybir.dt.float32)
                    nc.sync.dma_start(out=z32, in_=z_r[g])
                    nc.gpsimd.tensor_copy(out=z16, in_=z32)
                else:
                    # SWDGE (gpsimd) DMA: casts fp32 -> fp16 in flight and
                    # keeps the steady-state input stream off the sync queue
                    # so output DMAs can't head-of-line block it.
                    nc.gpsimd.dma_start(out=z16, in_=z_r[g])

                # Transpose to zT[p=d % 128, kt, n] (fp16, 1 cycle/row on PE)
                zT = zt_pool.tile([P, K_TILES, MAX_NODES], mybir.dt.float16)
                for kt in range(K_TILES):
                    ps_t = psum_t_pool.tile([P, MAX_NODES], mybir.dt.float16)
                    for t in range(N_TILES):
                        nc.tensor.transpose(
                            ps_t[:, t * P:(t + 1) * P],
                            z16[:, t, kt * P:(kt + 1) * P],
                            identity,
                        )
                    nc.vector.tensor_copy(out=zT[:, kt, :], in_=ps_t)

                # Two halves of 2 m-tiles each => [128, 1024] PSUM tiles,
                # each sigmoid'd and DMA'd out independently (0.5 MB chunks
                # keep the sync ring dense and start the write stream early).
                for h in range(2):
                    mm_ps = psum_mm_pool.tile([P, 2 * MAX_NODES], mybir.dt.float32)
                    for mi in range(2):
                        m = 2 * h + mi
                        for kt in range(K_TILES):
                            nc.tensor.matmul(
                                mm_ps[:, mi * MAX_NODES:(mi + 1) * MAX_NODES],
                                lhsT=zT[:, kt, m * P:(m + 1) * P],
                                rhs=zT[:, kt, :],
                                start=(kt == 0),
                                stop=(kt == K_TILES - 1),
                            )
                    o_t = out_pool.tile([P, 2, MAX_NODES], mybir.dt.float32)
                    nc.scalar.activation(
                        out=o_t.rearrange("p m n -> p (m n)"),
                        in_=mm_ps,
                        func=mybir.ActivationFunctionType.Sigmoid,
                    )
                    nc.sync.dma_start(out=out_r[g, h], in_=o_t)

    nc.compile()
    return nc


def _get_nc():
    global _NC
    if _NC is None:
        _NC = _build_bass()
    return _NC


def kernel(z, batch, num_graphs, max_nodes):
    global _last_results
    z = np.ascontiguousarray(np.asarray(z), dtype=np.float32)
    batch = np.asarray(batch)
    G = int(num_graphs)
    N = int(max_nodes)
    n_total, d = z.shape
    assert (G, N, d, n_total) == (NUM_GRAPHS, MAX_NODES, LATENT_DIM,
                                  NUM_GRAPHS * MAX_NODES), "hardcoded shapes"

    # Fast path: every graph has exactly max_nodes contiguous nodes.
    expected_batch = (np.arange(n_total) // N).astype(batch.dtype)
    dense = np.array_equal(batch, expected_batch)
    if dense:
        z_full = z
        mask2d = None
    else:
        # General ragged path: scatter into zero-padded [G, N, d] on host,
        # run the same device kernel, then zero out masked positions.
        counts = np.bincount(batch, minlength=G)
        starts = np.concatenate([[0], np.cumsum(counts)[:-1]])
        pos = np.arange(n_total) - starts[batch]
        z_pad = np.zeros((G, N, d), np.float32)
        valid = np.zeros((G, N), bool)
        z_pad[batch, pos] = z
        valid[batch, pos] = True
        z_full = z_pad.reshape(G * N, d)
        mask2d = valid[:, :, None] & valid[:, None, :]

    nc = _get_nc()
    rows = G_PER_CORE * MAX_NODES
    in_maps = [
        {"z": z_full[c * rows:(c + 1) * rows]} for c in range(N_CORES)
    ]
    _last_results = run_bass_kernel_spmd(
        nc, in_maps, core_ids=list(range(N_CORES))
    )
    out = np.concatenate(
        [r["out"] for r in _last_results.results], axis=0
    ).astype(np.float32)

    if mask2d is not None:
        out = np.where(mask2d, out, np.float32(0.0))
    return out
